# revision 55
# baseline (speedup 1.0000x reference)
"""Causal self-attention (B=4, T=2048, C=1024, H=16) on 8 TRN2 NeuronCores.

Sharding: core c -> (batch b = c//2, head-group g = c%2). Each core computes
QKV for its 8 heads of one batch, causal attention, and a partial output
projection (its heads' slice of W_proj). The pairwise reduction over head
groups (the "all-reduce after c_proj") plus b_proj is done on host at
gather time.

v7 — software-pipelined emission (375us -> 306us measured at full clock):
  * stage-1 (QKV) of chunk i+1 and proj of chunk i-1 are emitted as PE
    "filler" matmuls interleaved into the attention kb-loop of chunk i, so
    the in-order PE queue never waits on the softmax exp (ScalarE) and the
    HAM clock gate stays warm across phase boundaries (one K=8/8 span over
    the whole kernel vs 5 cold windows before).
  * the two heads of a pair share one [128, 2, 512] PSUM S-tile (2 banks);
    exp is ONE ScalarE op per kb (halves the per-op PSUM-access overhead),
    the boundary mask is ONE GpSimd op.
  * softmax denominators 1/x = exp(-ln(x)): the two rows are gathered by
    quick DVE copies so ONE Ln + ONE Exp on ScalarE covers both; the
    reciprocal row pair is broadcast with a bf16 (not fp32) matmul.
  * yT output is bf16 (halves output DMA); host upcasts, sums the two
    head-group partials and adds b_proj.
  * startup: only pair-0's chunk-0 stage-1 groups run eagerly (Q m=0,
    K m=4, V); the rest drain as gated fillers inside attn(0), and the
    wqk DMA is m-major in consumption order, so attention exp starts
    ~15us earlier while the input DMA is still streaming.
PSUM budget: 2x[128,2,512] (S, 4 banks) + 2x[65,512] (PV accum, 2 banks)
  + 2x[128,512] shared stage1/proj/recb (2 banks) = 8 banks.

Negative results (measured, do not retry):
  * K=64 row-strip-packed S matmuls (2 concurrent per pair): the doubled
    PE switching power trips the chip-level power throttle; EVERY engine
    downclocks ~20% (MM 380->455ns). Zero-padded K=128 wins on power.
  * DVE reciprocal(s) for the denominators: 3.35us latency each holds the
    yac PSUM banks across the pair handover and stalls the next pair's PV
    accumulation (~50us wall).
  * reciprocal_approx_fast custom-DVE op: "ISA wrong length" codegen bug
    in this walrus build.
  * 32KB wqk DMA chunks on the scalar/gpsimd trigger queues: sequencer
    trigger costs (~600ns each) delay attention exps / xt prefetches.
    (The m-major 256KB slicing above gets the benefit without them.)
"""

import sys
from collections import deque

for _p in ("/opt/trn_rl_repo", "/root/.axon_site/_ro/trn_rl_repo"):
    if _p not in sys.path:
        sys.path.insert(0, _p)

import ml_dtypes
import numpy as np

import concourse.bass as bass
import concourse.mybir as mybir
import concourse.tile as tile
from concourse.bass import ts
from concourse.bass_utils import run_bass_kernel_spmd

B, T, C, H, HD = 4, 2048, 1024, 16, 64
NH = 8           # heads per core
P = 128
QC = 512         # q-chunk width
NQC = T // QC    # 4
NKB = T // P     # 16 k-blocks
KO = C // P      # 8 contraction tiles for the C-dim
F32 = mybir.dt.float32
BF16 = mybir.dt.bfloat16


def build_nc():
    nc = bass.Bass()

    xT = nc.dram_tensor("xT", [C, T], BF16, kind="ExternalInput")
    Wqk = nc.dram_tensor("Wqk", [C, 2 * NH * HD], BF16, kind="ExternalInput")
    Wv = nc.dram_tensor("Wv", [C, NH * HD], BF16, kind="ExternalInput")
    Wp = nc.dram_tensor("Wp", [NH * HD, C], BF16, kind="ExternalInput")
    bqk = nc.dram_tensor("bqk", [P, 2 * NH * HD // P], F32, kind="ExternalInput")
    bv = nc.dram_tensor("bv", [NH * HD], F32, kind="ExternalInput")
    mask = nc.dram_tensor("mask", [P, P], BF16, kind="ExternalInput")
    yT = nc.dram_tensor("yT", [C, T], BF16, kind="ExternalOutput")

    xT_t = xT[:].rearrange("(ko p) t -> p ko t", p=P)        # [128, 8, T]
    yT_t = yT[:].rearrange("(mo p) t -> p mo t", p=P)        # [128, 8, T]
    Wqk_t = Wqk[:].rearrange("(ko p) n -> p ko n", p=P)      # [128, 8, 1024]
    Wv_t = Wv[:].rearrange("(ko p) n -> p ko n", p=P)        # [128, 8, 512]
    Wp_t = Wp[:].rearrange("(ko p) n -> p ko n", p=P)        # [128, 4, 1024]

    with tile.TileContext(nc) as tc:
        with (
            tc.tile_pool(name="consts", bufs=1) as consts,
            tc.tile_pool(name="persist", bufs=1) as persist,
            tc.tile_pool(name="w1", bufs=1) as w1pool,
            tc.tile_pool(name="xt", bufs=2) as xtpool,
            tc.tile_pool(name="qt", bufs=2) as qtpool,
            tc.tile_pool(name="yt", bufs=3) as ytpool,
            tc.tile_pool(name="pt", bufs=3) as ptpool,
            tc.tile_pool(name="yu", bufs=2) as yupool,
            tc.tile_pool(name="st", bufs=2) as stpool,
            tc.tile_pool(name="ps_ab", bufs=2, space="PSUM") as ps_ab,
            tc.tile_pool(name="ps_y", bufs=2, space="PSUM") as ps_y,
            tc.tile_pool(name="ps_w", bufs=2, space="PSUM") as ps_w,
        ):
            # ---- stage-1 weights, m-major (one 256KB DMA per output
            # m-slice, covering all k-tiles) in the exact order the eager
            # pair-0 prefix then the gated fillers consume them ----
            bqk_sb = consts.tile([P, 2 * NH * HD // P], F32)      # [128, 8]
            nc.sync.dma_start(bqk_sb[:], bqk[:])
            bv_sb = consts.tile([P, NH * HD], F32)                # [128, 512]
            nc.sync.dma_start(bv_sb[:], bass.AP(bv, 0, [[0, P], [1, NH * HD]]))
            wqk_sb = w1pool.tile([P, KO, 2 * NH * HD], BF16)       # 2MB
            wv_sb = w1pool.tile([P, KO, NH * HD], BF16)            # 1MB
            for m in (0, 4):
                nc.sync.dma_start(wqk_sb[:, :, ts(m, P)], Wqk_t[:, :, ts(m, P)])
            for k in range(KO):
                nc.sync.dma_start(wv_sb[:, k, :], Wv_t[:, k, :])
            mask2_sb = consts.tile([P, 2, P], BF16)               # j >= p, x2
            nc.sync.dma_start(mask2_sb[:, 0, :], mask[:])
            nc.sync.dma_start(mask2_sb[:, 1, :], mask[:])
            for m in (1, 5, 2, 6, 3, 7):
                nc.sync.dma_start(wqk_sb[:, :, ts(m, P)], Wqk_t[:, :, ts(m, P)])

            # recb broadcast stationary: col j reads rzf row 0 (head A) for
            # j < 64, row 64 (head B) for j >= 64. (Partition bases must be
            # 0/32/64/96, so the two reciprocal rows live at 0 and 64.)
            ones2 = consts.tile([HD + 1, P], BF16)
            nc.vector.memset(ones2[:], 0.0)
            nc.vector.memset(ones2[0:1, 0:HD], 1.0)
            nc.vector.memset(ones2[HD : HD + 1, HD:P], 1.0)
            # ping-pong denominator tiles. 1/x is computed as exp(-ln(x)) on
            # ScalarE (Exp+Ln share an activation table, and the custom-DVE
            # reciprocal trips a codegen bug in this walrus build). lr rows
            # 1..63 stay zero -> exp(-0)=1 in rzb, which ones2's zero rows
            # ignore.
            lrs, rzbs, rgs = [], [], []
            for _i in range(2):
                lr = consts.tile([HD + 1, QC], F32, tag=f"lr{_i}")
                nc.vector.memset(lr[:], 0.0)
                lrs.append(lr)
                rzb = consts.tile([HD + 1, QC], BF16, tag=f"rzb{_i}")
                rzbs.append(rzb)
                # denominator-row gather target; rows 1..63 stay 1.0 so the
                # merged Ln sees ln(1)=0 there (never -inf)
                rg = consts.tile([HD + 1, QC], F32, tag=f"rg{_i}")
                nc.vector.memset(rg[:], 1.0)
                rgs.append(rg)

            # ---- persistent activations ----
            # kt_pad keeps each head's K^T zero-padded to the full 128
            # contraction partitions. Row-strip-packed K=64 S matmuls were
            # tried and REGRESSED ~17%: two concurrent real-data strips
            # double the PE's switching power, the chip drops into the P0
            # power state and EVERY engine downclocks ~20%. Multiplying
            # into zeros is nearly free power-wise, so the padded layout
            # wins despite "wasting" half the array.
            kt_pad = persist.tile([P, NH, T], BF16)                # 4MB
            nc.vector.memset(kt_pad[HD:P, 0:NH:2, :], 0.0)
            nc.vector.memset(kt_pad[0:HD, 1:NH:2, :], 0.0)
            vex = persist.tile([P, NKB, NH, HD + 1], BF16)         # 2.1MB
            nc.vector.memset(vex[:, :, :, HD:], 1.0)

            # Wp is first needed at proj(0), deep into attn(1): load last
            wp_sb = consts.tile([P, NH * HD // P, C], BF16)        # [128, 4, 1024]
            nc.sync.dma_start(wp_sb[:], Wp_t[:])

            qts = [None] * NQC
            ytqs = [None] * NQC

            def prefetch_xt(tc_i):
                xt = xtpool.tile([P, KO, QC], BF16, tag="xt")
                for k in range(KO):
                    nc.gpsimd.dma_start(xt[:, k, :], xT_t[:, k, ts(tc_i, QC)])
                return xt

            def stage1_closures(tc_i, xt):
                """One closure per PE matmul; the last closure of each
                m-group also emits the psum read-out (DVE)."""
                qt = qtpool.tile([P, NH * HD // P, QC], BF16, tag="qt",
                                 name=f"qt{tc_i}")
                qts[tc_i] = qt
                fillers = []
                state = {}

                def qk_mm(m, k):
                    def run():
                        if k == 0:
                            state["ps"] = ps_w.tile([P, QC], F32, tag="w",
                                                    name="s1ps")
                        ps = state["ps"]
                        nc.tensor.matmul(
                            ps[:], wqk_sb[:, k, ts(m, P)], xt[:, k, :],
                            start=(k == 0), stop=(k == KO - 1),
                        )
                        if k == KO - 1:
                            if m < NH * HD // P:
                                nc.vector.tensor_scalar_add(
                                    qt[:, m, :], ps[:], bqk_sb[:, m : m + 1]
                                )
                            else:
                                mk = m - NH * HD // P
                                hA, hB = 2 * mk, 2 * mk + 1
                                tsl = ts(tc_i, QC)
                                nc.vector.tensor_scalar_add(
                                    kt_pad[0:HD, hA, tsl], ps[0:HD, :],
                                    bqk_sb[0:HD, m : m + 1],
                                )
                                nc.vector.tensor_scalar_add(
                                    kt_pad[HD:P, hB, tsl], ps[HD:P, :],
                                    bqk_sb[HD:P, m : m + 1],
                                )
                    return run

                def v_mm(t4, k):
                    def run():
                        if k == 0:
                            state["ps"] = ps_w.tile([P, NH * HD], F32,
                                                    tag="w", name="s1pv")
                        psv = state["ps"]
                        nc.tensor.matmul(
                            psv[:], xt[:, k, ts(t4, P)], wv_sb[:, k, :],
                            start=(k == 0), stop=(k == KO - 1),
                        )
                        if k == KO - 1:
                            kb = tc_i * (QC // P) + t4
                            nc.vector.tensor_add(
                                vex[:, kb, :, :HD],
                                psv[:].rearrange("p (h d) -> p h d", h=NH),
                                bv_sb[:].rearrange("p (h d) -> p h d", h=NH),
                            )
                    return run

                for m in range(2 * NH * HD // P):   # 8: m<4 Q, m>=4 K
                    for k in range(KO):
                        fillers.append(qk_mm(m, k))
                for t4 in range(QC // P):
                    for k in range(KO):
                        fillers.append(v_mm(t4, k))
                return fillers

            def proj_closures(qc):
                """Projection of chunk qc; consumes ytqs[qc] (normed)."""
                ytq = ytqs[qc]
                fillers = []
                state = {}
                KK = NH * HD // P   # 4

                def p_mm(m, kk):
                    def run():
                        if kk == 0:
                            state["ps"] = ps_w.tile([P, QC], F32, tag="w",
                                                    name="prps")
                        pp = state["ps"]
                        nc.tensor.matmul(
                            pp[:], wp_sb[:, kk, ts(m, P)], ytq[:, kk, :],
                            start=(kk == 0), stop=(kk == KK - 1),
                        )
                        if kk == KK - 1:
                            st = stpool.tile([P, QC], BF16, tag="st",
                                             name="stt")
                            nc.vector.tensor_copy(st[:], pp[:])
                            nc.sync.dma_start(yT_t[:, m, ts(qc, QC)], st[:])
                    return run

                for m in range(C // P):
                    for kk in range(KK):
                        fillers.append(p_mm(m, kk))
                return fillers

            s0_q = deque()     # chunk-0 stage-1 leftovers (gated per pair)
            must_q = deque()   # stage-1: must drain within current phase
            lazy_q = deque()   # proj: drain opportunistically
            # during chunk 3 (ACT-bound, PE-slack) hold a few proj closures
            # back until the LAST pair, whose iterations otherwise leave the
            # PE with nothing to do while ScalarE finishes the final exps
            lazy_floor = [0]

            def drain(n):
                for _ in range(n):
                    if s0_q:
                        s0_q.popleft()()
                    elif must_q:
                        must_q.popleft()()
                    elif lazy_q and len(lazy_q) > lazy_floor[0]:
                        lazy_q.popleft()()
                    else:
                        break

            # softmax-normalization deferral (global across chunks): the
            # recb matmul + norm muls of pair i are emitted at the end of
            # pair i+1, so the in-order PE queue never waits on the
            # reciprocal chain.
            pending = [None]

            def emit_norm():
                if pending[0] is None:
                    return
                qc_p, ytq, g, yu, rzb = pending[0]
                pending[0] = None
                recb = ps_w.tile([P, QC], F32, tag="w", name="recb")
                nc.tensor.matmul(recb[:], ones2[:], rzb[:], start=True,
                                 stop=True)
                nc.vector.tensor_mul(ytq[0:HD, g, :], yu[0:HD, :],
                                     recb[0:HD, :])
                nc.vector.tensor_mul(ytq[HD:P, g, :], yu[HD:P, :],
                                     recb[HD:P, :])
                if g == NH // 2 - 1:
                    # chunk qc_p's ytq is now fully written (in emission
                    # order) -> its projection may enter the filler queue.
                    # Pushing it any earlier would let proj matmuls be
                    # emitted before this norm, erasing the dependency.
                    lazy_q.extend(proj_closures(qc_p))

            # ---------- stage-1 of chunk 0: only pair-0's groups (Q m=0,
            # K m=4, all V blocks) run eagerly; the rest drain as gated
            # fillers inside attn(0) so attention/exp starts ~15us earlier,
            # overlapping the input-DMA trickle ----------
            xt0 = prefetch_xt(0)
            s1_0 = stage1_closures(0, xt0)
            for i in list(range(0, 8)) + list(range(32, 40)) + list(
                range(64, 96)
            ):
                s1_0[i]()
            for m in (1, 5, 2, 6, 3, 7):   # DMA arrival order
                s0_q.extend(s1_0[8 * m : 8 * m + 8])

            for qc in range(NQC):
                if qc + 1 < NQC and qc > 0:
                    # (for qc==0 this happens at the pair-1 gate instead:
                    # the 1MB xt(1) prefetch must not compete with the
                    # wv/wqk stream in the 15-23us startup DMA pinch)
                    xt_n = prefetch_xt(qc + 1)
                    must_q.extend(stage1_closures(qc + 1, xt_n))
                if qc == NQC - 1:
                    lazy_floor[0] = 12

                qt = qts[qc]
                ytq = ytpool.tile([P, NH * HD // P, QC], BF16, tag="ytq",
                                  name=f"ytq{qc}")
                ytqs[qc] = ytq
                nkb = (qc + 1) * (QC // P)
                total_iters = (NH // 2) * nkb
                it = 0

                for g in range(NH // 2):  # head pairs
                    if qc == 0 and g > 0:
                        # airtight gate: pair g's Q/K stage-1 groups must be
                        # EMITTED before its S matmuls, or the dependency
                        # doesn't exist and S reads garbage
                        while len(s0_q) > 48 - 16 * g:
                            drain(1)
                        if g == 1:
                            xt_n = prefetch_xt(1)
                            must_q.extend(stage1_closures(1, xt_n))
                    if qc == NQC - 1 and g == NH // 2 - 1:
                        lazy_floor[0] = 0   # release the reserve
                    hA, hB = 2 * g, 2 * g + 1
                    yacA = ps_y.tile([HD + 1, QC], F32, tag="y", name="yacA")
                    yacB = ps_y.tile([HD + 1, QC], F32, tag="y", name="yacB")
                    prev = None
                    for kb in range(nkb):
                        d = kb - qc * (QC // P)
                        off = 0 if d < 0 else d * P
                        w = QC - off
                        sps = ps_ab.tile([P, 2, QC], F32, tag="ab",
                                         name="sps")
                        nc.tensor.matmul(
                            sps[:, 0, :w], kt_pad[:, hA, ts(kb, P)],
                            qt[:, g, off:QC], start=True, stop=True,
                        )
                        nc.tensor.matmul(
                            sps[:, 1, :w], kt_pad[:, hB, ts(kb, P)],
                            qt[:, g, off:QC], start=True, stop=True,
                        )
                        pt = ptpool.tile([P, 2, QC], BF16, tag="pt",
                                         name="pt")
                        nc.scalar.activation(
                            pt[:, :, :w], sps[:, :, :w],
                            mybir.ActivationFunctionType.Exp,
                            scale=1.0 / np.sqrt(HD),
                        )
                        if d >= 0:  # boundary 128 cols get the j>=p mask
                            nc.gpsimd.tensor_mul(
                                pt[:, :, :P], pt[:, :, :P], mask2_sb[:]
                            )
                        # pace the fillers: keep the PE busy while exp runs
                        iters_left = total_iters - it
                        rem = len(s0_q) + len(must_q)
                        need = -(-rem // iters_left)  # ceil
                        n = max(3 if d >= 0 else 2, need)
                        if kb <= 1:
                            # extra cover for the yac-bank handover from the
                            # previous pair's reciprocal readers
                            n = max(n, 4)
                        if it >= 2:
                            drain(n)
                        it += 1
                        if prev is not None:
                            pkb, ppt, poff, pw = prev
                            nc.tensor.matmul(
                                yacA[:, poff:QC], vex[:, pkb, hA, :],
                                ppt[:, 0, :pw],
                                start=(pkb == 0), stop=(pkb == nkb - 1),
                            )
                            nc.tensor.matmul(
                                yacB[:, poff:QC], vex[:, pkb, hB, :],
                                ppt[:, 1, :pw],
                                start=(pkb == 0), stop=(pkb == nkb - 1),
                            )
                        prev = (kb, pt, off, w)
                    drain(3)
                    pkb, ppt, poff, pw = prev
                    nc.tensor.matmul(
                        yacA[:, poff:QC], vex[:, pkb, hA, :], ppt[:, 0, :pw],
                        start=(pkb == 0), stop=(pkb == nkb - 1),
                    )
                    nc.tensor.matmul(
                        yacB[:, poff:QC], vex[:, pkb, hB, :], ppt[:, 1, :pw],
                        start=(pkb == 0), stop=(pkb == nkb - 1),
                    )
                    # stage unnormalized y to SBUF (frees the yac banks) and
                    # kick off the pair's denominator reciprocals
                    yu = yupool.tile([P, QC], BF16, tag="yu", name="yu")
                    nc.vector.tensor_copy(yu[0:HD, :], yacA[:HD, :])
                    nc.vector.tensor_copy(yu[HD:P, :], yacB[:HD, :])
                    # 1/x = exp(-ln(x)) on ScalarE, with the two denominator
                    # rows first gathered into one tile by quick DVE copies
                    # so a SINGLE Ln covers both (ScalarE is the bottleneck
                    # in late chunks). A DVE-reciprocal variant was tried
                    # and REGRESSED ~50us: its 3.35us latency holds the yac
                    # PSUM banks hostage, stalling the next pair's PV
                    # matmuls on the bank handover; the copies here are
                    # ~600ns so yac frees almost immediately.
                    lr, rzb, rg = lrs[g % 2], rzbs[g % 2], rgs[g % 2]
                    nc.vector.tensor_copy(rg[0:1, :], yacA[HD : HD + 1, :])
                    nc.vector.tensor_copy(
                        rg[HD : HD + 1, :], yacB[HD : HD + 1, :]
                    )
                    nc.scalar.activation(
                        lr[:], rg[:], mybir.ActivationFunctionType.Ln,
                    )
                    nc.scalar.activation(
                        rzb[:], lr[:],
                        mybir.ActivationFunctionType.Exp, scale=-1.0,
                    )
                    emit_norm()
                    pending[0] = (qc, ytq, g, yu, rzb)

                # force-drain any stage-1 leftovers before the next chunk's
                # attention needs qt/kt/vex. The last pair's normalization
                # stays pending across the chunk boundary; emit_norm pushes
                # the chunk's proj closures once it fires.
                drain(len(s0_q) + len(must_q))

            emit_norm()
            drain(len(s0_q) + len(must_q) + len(lazy_q))

    return nc


def legalize_waits(nc):
    """This walrus build accepts at most 1 sync wait per instruction (0 for
    self-loading fp32/fp32r Matmult, whose LW slot takes none). Move excess
    waits onto preceding same-engine NoOps; engines execute in order so the
    guarantee is identical."""
    n = 0
    for blk in nc.m.functions[0].blocks:
        new = []
        for inst in blk.instructions:
            si = inst.sync_info
            waits = list(si.on_wait) if si is not None and si.on_wait else []
            lim = 0 if inst.opcode in ("Matmult", "Ldweights") else 1
            if len(waits) > lim:
                keep = waits[len(waits) - lim:] if lim else []
                for w in waits[: len(waits) - lim]:
                    n += 1
                    new.append(mybir.InstNoOp(
                        name=f"I-wfix{n}", engine=inst.engine, ins=[], outs=[],
                        sync_info=mybir.SyncInfo(on_wait=[w], on_update=[]),
                    ))
                inst.sync_info = mybir.SyncInfo(
                    on_wait=keep,
                    on_update=list(si.on_update) if si.on_update else [],
                )
            new.append(inst)
        blk.instructions = new
    return n


def _host_inputs(x, W_attn, b_attn, W_proj):
    """Build the 8 per-core input maps."""
    kl = np.arange(P)[:, None]
    ql = np.arange(P)[None, :]
    mask = (ql >= kl).astype(ml_dtypes.bfloat16)  # [128, 128]

    in_maps = []
    for core in range(8):
        b, g = core // 2, core % 2
        qs = slice(g * NH * HD, (g + 1) * NH * HD)
        ks = slice(C + g * NH * HD, C + (g + 1) * NH * HD)
        vs = slice(2 * C + g * NH * HD, 2 * C + (g + 1) * NH * HD)
        wqk = np.ascontiguousarray(
            np.concatenate([W_attn[:, qs], W_attn[:, ks]], axis=1)
        )
        bqk = (
            np.concatenate([b_attn[qs], b_attn[ks]])
            .reshape(2 * NH * HD // P, P)
            .T.copy()
        )
        in_maps.append(
            {
                "xT": np.ascontiguousarray(x[b].T).astype(ml_dtypes.bfloat16),
                "Wqk": wqk.astype(ml_dtypes.bfloat16),
                "Wv": np.ascontiguousarray(W_attn[:, vs]).astype(
                    ml_dtypes.bfloat16
                ),
                "Wp": np.ascontiguousarray(
                    W_proj[g * NH * HD : (g + 1) * NH * HD]
                ).astype(ml_dtypes.bfloat16),
                "bqk": np.ascontiguousarray(bqk),
                "bv": np.ascontiguousarray(b_attn[vs]),
                "mask": mask,
            }
        )
    return in_maps


def run(x, W_attn, b_attn, W_proj, b_proj, trace=False):
    """Returns (y, BassKernelResults)."""
    x = np.asarray(x, dtype=np.float32)
    W_attn = np.asarray(W_attn, dtype=np.float32)
    b_attn = np.asarray(b_attn, dtype=np.float32)
    W_proj = np.asarray(W_proj, dtype=np.float32)
    b_proj = np.asarray(b_proj, dtype=np.float32)

    nc = build_nc()
    legalize_waits(nc)
    in_maps = _host_inputs(x, W_attn, b_attn, W_proj)
    res = run_bass_kernel_spmd(nc, in_maps, list(range(8)), trace=trace)

    y = np.empty((B, T, C), dtype=np.float32)
    for b in range(B):
        acc = (
            res.results[2 * b]["yT"].astype(np.float32)
            + res.results[2 * b + 1]["yT"].astype(np.float32)
        )
        y[b] = acc.T + b_proj
    return y, res


def kernel(x, W_attn, b_attn, W_proj, b_proj):
    y, _ = run(x, W_attn, b_attn, W_proj, b_proj)
    return y


# revision 59
# speedup vs baseline: 1.0066x; 1.0066x over previous
"""Causal self-attention (B=4, T=2048, C=1024, H=16) on 8 TRN2 NeuronCores.

Sharding: core c -> (batch b = c//2, head-group g = c%2). Each core computes
QKV for its 8 heads of one batch, causal attention, and a partial output
projection (its heads' slice of W_proj). The pairwise reduction over head
groups (the "all-reduce after c_proj") plus b_proj is done on host at
gather time.

v7 — software-pipelined emission (375us -> 306us measured at full clock):
  * stage-1 (QKV) of chunk i+1 and proj of chunk i-1 are emitted as PE
    "filler" matmuls interleaved into the attention kb-loop of chunk i, so
    the in-order PE queue never waits on the softmax exp (ScalarE) and the
    HAM clock gate stays warm across phase boundaries (one K=8/8 span over
    the whole kernel vs 5 cold windows before).
  * the two heads of a pair share one [128, 2, 512] PSUM S-tile (2 banks);
    exp is ONE ScalarE op per kb (halves the per-op PSUM-access overhead),
    the boundary mask is ONE GpSimd op.
  * softmax denominators 1/x = exp(-ln(x)): the two rows are gathered by
    quick DVE copies so ONE Ln + ONE Exp on ScalarE covers both; the
    reciprocal row pair is broadcast with a bf16 (not fp32) matmul.
  * yT output is bf16 (halves output DMA); host upcasts, sums the two
    head-group partials and adds b_proj.
  * startup: only pair-0's chunk-0 stage-1 groups run eagerly (Q m=0,
    K m=4, V); the rest drain as gated fillers inside attn(0), and the
    wqk DMA is m-major in consumption order, so attention exp starts
    ~15us earlier while the input DMA is still streaming.
PSUM budget: 2x[128,2,512] (S, 4 banks) + 2x[65,512] (PV accum, 2 banks)
  + 2x[128,512] shared stage1/proj/recb (2 banks) = 8 banks.

Negative results (measured, do not retry):
  * K=64 row-strip-packed S matmuls (2 concurrent per pair): the doubled
    PE switching power trips the chip-level power throttle; EVERY engine
    downclocks ~20% (MM 380->455ns). Zero-padded K=128 wins on power.
  * DVE reciprocal(s) for the denominators: 3.35us latency each holds the
    yac PSUM banks across the pair handover and stalls the next pair's PV
    accumulation (~50us wall).
  * reciprocal_approx_fast custom-DVE op: "ISA wrong length" codegen bug
    in this walrus build.
  * 32KB wqk DMA chunks on the scalar/gpsimd trigger queues: sequencer
    trigger costs (~600ns each) delay attention exps / xt prefetches.
    (The m-major 256KB slicing above gets the benefit without them.)
  * SBUF->SBUF DMA row-broadcast of the reciprocal rows (to replace the
    recb matmul): both AP.partition_broadcast and a manual 0-stride
    partition AP panic in this bass build's DMACopy lowering.
  * tail/prefetch schedule shuffles (proj-closure reserves, deferred
    xt(1) prefetch, pt bufs=4): all within +/-1.2us run noise of 306.5us
    (measured 307.1 / 307.7); the schedule is at a plateau where the
    residual PE idle is runtime preamble, DMA-bandwidth-bound startup,
    the serial final-normalization tail, and distributed semaphore slack.
"""

import sys
from collections import deque

for _p in ("/opt/trn_rl_repo", "/root/.axon_site/_ro/trn_rl_repo"):
    if _p not in sys.path:
        sys.path.insert(0, _p)

import ml_dtypes
import numpy as np

import concourse.bass as bass
import concourse.mybir as mybir
import concourse.tile as tile
from concourse.bass import ts
from concourse.bass_utils import run_bass_kernel_spmd

B, T, C, H, HD = 4, 2048, 1024, 16, 64
NH = 8           # heads per core
P = 128
QC = 512         # q-chunk width
NQC = T // QC    # 4
NKB = T // P     # 16 k-blocks
KO = C // P      # 8 contraction tiles for the C-dim
F32 = mybir.dt.float32
BF16 = mybir.dt.bfloat16


def build_nc():
    nc = bass.Bass()

    xT = nc.dram_tensor("xT", [C, T], BF16, kind="ExternalInput")
    Wqk = nc.dram_tensor("Wqk", [C, 2 * NH * HD], BF16, kind="ExternalInput")
    Wv = nc.dram_tensor("Wv", [C, NH * HD], BF16, kind="ExternalInput")
    Wp = nc.dram_tensor("Wp", [NH * HD, C], BF16, kind="ExternalInput")
    bqk = nc.dram_tensor("bqk", [P, 2 * NH * HD // P], F32, kind="ExternalInput")
    bv = nc.dram_tensor("bv", [NH * HD], F32, kind="ExternalInput")
    mask = nc.dram_tensor("mask", [P, P], BF16, kind="ExternalInput")
    yT = nc.dram_tensor("yT", [C, T], BF16, kind="ExternalOutput")

    xT_t = xT[:].rearrange("(ko p) t -> p ko t", p=P)        # [128, 8, T]
    yT_t = yT[:].rearrange("(mo p) t -> p mo t", p=P)        # [128, 8, T]
    Wqk_t = Wqk[:].rearrange("(ko p) n -> p ko n", p=P)      # [128, 8, 1024]
    Wv_t = Wv[:].rearrange("(ko p) n -> p ko n", p=P)        # [128, 8, 512]
    Wp_t = Wp[:].rearrange("(ko p) n -> p ko n", p=P)        # [128, 4, 1024]

    with tile.TileContext(nc) as tc:
        with (
            tc.tile_pool(name="consts", bufs=1) as consts,
            tc.tile_pool(name="persist", bufs=1) as persist,
            tc.tile_pool(name="w1", bufs=1) as w1pool,
            tc.tile_pool(name="xt", bufs=2) as xtpool,
            tc.tile_pool(name="qt", bufs=2) as qtpool,
            tc.tile_pool(name="yt", bufs=3) as ytpool,
            tc.tile_pool(name="pt", bufs=3) as ptpool,
            tc.tile_pool(name="yu", bufs=2) as yupool,
            tc.tile_pool(name="st", bufs=2) as stpool,
            tc.tile_pool(name="ps_ab", bufs=2, space="PSUM") as ps_ab,
            tc.tile_pool(name="ps_y", bufs=2, space="PSUM") as ps_y,
            tc.tile_pool(name="ps_w", bufs=2, space="PSUM") as ps_w,
        ):
            # ---- stage-1 weights, m-major (one 256KB DMA per output
            # m-slice, covering all k-tiles) in the exact order the eager
            # pair-0 prefix then the gated fillers consume them ----
            bqk_sb = consts.tile([P, 2 * NH * HD // P], F32)      # [128, 8]
            nc.sync.dma_start(bqk_sb[:], bqk[:])
            bv_sb = consts.tile([P, NH * HD], F32)                # [128, 512]
            nc.sync.dma_start(bv_sb[:], bass.AP(bv, 0, [[0, P], [1, NH * HD]]))
            wqk_sb = w1pool.tile([P, KO, 2 * NH * HD], BF16)       # 2MB
            wv_sb = w1pool.tile([P, KO, NH * HD], BF16)            # 1MB
            for m in (0, 4):
                nc.sync.dma_start(wqk_sb[:, :, ts(m, P)], Wqk_t[:, :, ts(m, P)])
            for k in range(KO):
                nc.sync.dma_start(wv_sb[:, k, :], Wv_t[:, k, :])
            mask2_sb = consts.tile([P, 2, P], BF16)               # j >= p, x2
            nc.sync.dma_start(mask2_sb[:, 0, :], mask[:])
            nc.sync.dma_start(mask2_sb[:, 1, :], mask[:])
            for m in (1, 5, 2, 6, 3, 7):
                nc.sync.dma_start(wqk_sb[:, :, ts(m, P)], Wqk_t[:, :, ts(m, P)])

            # recb broadcast stationary: col j reads rzf row 0 (head A) for
            # j < 64, row 64 (head B) for j >= 64. (Partition bases must be
            # 0/32/64/96, so the two reciprocal rows live at 0 and 64.)
            ones2 = consts.tile([HD + 1, P], BF16)
            nc.vector.memset(ones2[:], 0.0)
            nc.vector.memset(ones2[0:1, 0:HD], 1.0)
            nc.vector.memset(ones2[HD : HD + 1, HD:P], 1.0)
            # ping-pong denominator tiles. 1/x is computed as exp(-ln(x)) on
            # ScalarE (Exp+Ln share an activation table, and the custom-DVE
            # reciprocal trips a codegen bug in this walrus build). lr rows
            # 1..63 stay zero -> exp(-0)=1 in rzb, which ones2's zero rows
            # ignore.
            lrs, rzbs, rgs = [], [], []
            for _i in range(2):
                lr = consts.tile([HD + 1, QC], F32, tag=f"lr{_i}")
                nc.vector.memset(lr[:], 0.0)
                lrs.append(lr)
                rzb = consts.tile([HD + 1, QC], BF16, tag=f"rzb{_i}")
                rzbs.append(rzb)
                # denominator-row gather target; rows 1..63 stay 1.0 so the
                # merged Ln sees ln(1)=0 there (never -inf)
                rg = consts.tile([HD + 1, QC], F32, tag=f"rg{_i}")
                nc.vector.memset(rg[:], 1.0)
                rgs.append(rg)

            # ---- persistent activations ----
            # kt_pad keeps each head's K^T zero-padded to the full 128
            # contraction partitions. Row-strip-packed K=64 S matmuls were
            # tried and REGRESSED ~17%: two concurrent real-data strips
            # double the PE's switching power, the chip drops into the P0
            # power state and EVERY engine downclocks ~20%. Multiplying
            # into zeros is nearly free power-wise, so the padded layout
            # wins despite "wasting" half the array.
            kt_pad = persist.tile([P, NH, T], BF16)                # 4MB
            nc.vector.memset(kt_pad[HD:P, 0:NH:2, :], 0.0)
            nc.vector.memset(kt_pad[0:HD, 1:NH:2, :], 0.0)
            vex = persist.tile([P, NKB, NH, HD + 1], BF16)         # 2.1MB
            nc.vector.memset(vex[:, :, :, HD:], 1.0)

            # Wp is first needed at proj(0), deep into attn(1): load last
            wp_sb = consts.tile([P, NH * HD // P, C], BF16)        # [128, 4, 1024]
            nc.sync.dma_start(wp_sb[:], Wp_t[:])

            qts = [None] * NQC
            ytqs = [None] * NQC

            def prefetch_xt(tc_i):
                xt = xtpool.tile([P, KO, QC], BF16, tag="xt")
                for k in range(KO):
                    nc.gpsimd.dma_start(xt[:, k, :], xT_t[:, k, ts(tc_i, QC)])
                return xt

            def stage1_closures(tc_i, xt):
                """One closure per PE matmul; the last closure of each
                m-group also emits the psum read-out (DVE)."""
                qt = qtpool.tile([P, NH * HD // P, QC], BF16, tag="qt",
                                 name=f"qt{tc_i}")
                qts[tc_i] = qt
                fillers = []
                state = {}

                def qk_mm(m, k):
                    def run():
                        if k == 0:
                            state["ps"] = ps_w.tile([P, QC], F32, tag="w",
                                                    name="s1ps")
                        ps = state["ps"]
                        nc.tensor.matmul(
                            ps[:], wqk_sb[:, k, ts(m, P)], xt[:, k, :],
                            start=(k == 0), stop=(k == KO - 1),
                        )
                        if k == KO - 1:
                            if m < NH * HD // P:
                                nc.vector.tensor_scalar_add(
                                    qt[:, m, :], ps[:], bqk_sb[:, m : m + 1]
                                )
                            else:
                                mk = m - NH * HD // P
                                hA, hB = 2 * mk, 2 * mk + 1
                                tsl = ts(tc_i, QC)
                                nc.vector.tensor_scalar_add(
                                    kt_pad[0:HD, hA, tsl], ps[0:HD, :],
                                    bqk_sb[0:HD, m : m + 1],
                                )
                                nc.vector.tensor_scalar_add(
                                    kt_pad[HD:P, hB, tsl], ps[HD:P, :],
                                    bqk_sb[HD:P, m : m + 1],
                                )
                    return run

                def v_mm(t4, k):
                    def run():
                        if k == 0:
                            state["ps"] = ps_w.tile([P, NH * HD], F32,
                                                    tag="w", name="s1pv")
                        psv = state["ps"]
                        nc.tensor.matmul(
                            psv[:], xt[:, k, ts(t4, P)], wv_sb[:, k, :],
                            start=(k == 0), stop=(k == KO - 1),
                        )
                        if k == KO - 1:
                            kb = tc_i * (QC // P) + t4
                            nc.vector.tensor_add(
                                vex[:, kb, :, :HD],
                                psv[:].rearrange("p (h d) -> p h d", h=NH),
                                bv_sb[:].rearrange("p (h d) -> p h d", h=NH),
                            )
                    return run

                for m in range(2 * NH * HD // P):   # 8: m<4 Q, m>=4 K
                    for k in range(KO):
                        fillers.append(qk_mm(m, k))
                for t4 in range(QC // P):
                    for k in range(KO):
                        fillers.append(v_mm(t4, k))
                return fillers

            def proj_closures(qc):
                """Projection of chunk qc; consumes ytqs[qc] (normed)."""
                ytq = ytqs[qc]
                fillers = []
                state = {}
                KK = NH * HD // P   # 4

                def p_mm(m, kk):
                    def run():
                        if kk == 0:
                            state["ps"] = ps_w.tile([P, QC], F32, tag="w",
                                                    name="prps")
                        pp = state["ps"]
                        nc.tensor.matmul(
                            pp[:], wp_sb[:, kk, ts(m, P)], ytq[:, kk, :],
                            start=(kk == 0), stop=(kk == KK - 1),
                        )
                        if kk == KK - 1:
                            st = stpool.tile([P, QC], BF16, tag="st",
                                             name="stt")
                            nc.vector.tensor_copy(st[:], pp[:])
                            nc.sync.dma_start(yT_t[:, m, ts(qc, QC)], st[:])
                    return run

                for m in range(C // P):
                    for kk in range(KK):
                        fillers.append(p_mm(m, kk))
                return fillers

            s0_q = deque()     # chunk-0 stage-1 leftovers (gated per pair)
            must_q = deque()   # stage-1: must drain within current phase
            lazy_q = deque()   # proj: drain opportunistically

            def drain(n):
                for _ in range(n):
                    if s0_q:
                        s0_q.popleft()()
                    elif must_q:
                        must_q.popleft()()
                    elif lazy_q:
                        lazy_q.popleft()()
                    else:
                        break

            # softmax-normalization deferral (global across chunks): the
            # recb matmul + norm muls of pair i are emitted at the end of
            # pair i+1, so the in-order PE queue never waits on the
            # reciprocal chain.
            pending = [None]

            def emit_norm():
                if pending[0] is None:
                    return
                qc_p, ytq, g, yu, rzb = pending[0]
                pending[0] = None
                recb = ps_w.tile([P, QC], F32, tag="w", name="recb")
                nc.tensor.matmul(recb[:], ones2[:], rzb[:], start=True,
                                 stop=True)
                nc.vector.tensor_mul(ytq[0:HD, g, :], yu[0:HD, :],
                                     recb[0:HD, :])
                nc.vector.tensor_mul(ytq[HD:P, g, :], yu[HD:P, :],
                                     recb[HD:P, :])
                if g == NH // 2 - 1:
                    # chunk qc_p's ytq is now fully written (in emission
                    # order) -> its projection may enter the filler queue.
                    # Pushing it any earlier would let proj matmuls be
                    # emitted before this norm, erasing the dependency.
                    lazy_q.extend(proj_closures(qc_p))

            # ---------- stage-1 of chunk 0: only pair-0's groups (Q m=0,
            # K m=4, all V blocks) run eagerly; the rest drain as gated
            # fillers inside attn(0) so attention/exp starts ~15us earlier,
            # overlapping the input-DMA trickle ----------
            xt0 = prefetch_xt(0)
            s1_0 = stage1_closures(0, xt0)
            for i in list(range(0, 8)) + list(range(32, 40)) + list(
                range(64, 96)
            ):
                s1_0[i]()
            for m in (1, 5, 2, 6, 3, 7):   # DMA arrival order
                s0_q.extend(s1_0[8 * m : 8 * m + 8])

            for qc in range(NQC):
                if qc + 1 < NQC:
                    xt_n = prefetch_xt(qc + 1)
                    must_q.extend(stage1_closures(qc + 1, xt_n))

                qt = qts[qc]
                ytq = ytpool.tile([P, NH * HD // P, QC], BF16, tag="ytq",
                                  name=f"ytq{qc}")
                ytqs[qc] = ytq
                nkb = (qc + 1) * (QC // P)
                total_iters = (NH // 2) * nkb
                it = 0

                for g in range(NH // 2):  # head pairs
                    if qc == 0 and g > 0:
                        # airtight gate: pair g's Q/K stage-1 groups must be
                        # EMITTED before its S matmuls, or the dependency
                        # doesn't exist and S reads garbage
                        while len(s0_q) > 48 - 16 * g:
                            drain(1)
                    hA, hB = 2 * g, 2 * g + 1
                    yacA = ps_y.tile([HD + 1, QC], F32, tag="y", name="yacA")
                    yacB = ps_y.tile([HD + 1, QC], F32, tag="y", name="yacB")
                    prev = None
                    for kb in range(nkb):
                        d = kb - qc * (QC // P)
                        off = 0 if d < 0 else d * P
                        w = QC - off
                        sps = ps_ab.tile([P, 2, QC], F32, tag="ab",
                                         name="sps")
                        nc.tensor.matmul(
                            sps[:, 0, :w], kt_pad[:, hA, ts(kb, P)],
                            qt[:, g, off:QC], start=True, stop=True,
                        )
                        nc.tensor.matmul(
                            sps[:, 1, :w], kt_pad[:, hB, ts(kb, P)],
                            qt[:, g, off:QC], start=True, stop=True,
                        )
                        pt = ptpool.tile([P, 2, QC], BF16, tag="pt",
                                         name="pt")
                        nc.scalar.activation(
                            pt[:, :, :w], sps[:, :, :w],
                            mybir.ActivationFunctionType.Exp,
                            scale=1.0 / np.sqrt(HD),
                        )
                        if d >= 0:  # boundary 128 cols get the j>=p mask
                            nc.gpsimd.tensor_mul(
                                pt[:, :, :P], pt[:, :, :P], mask2_sb[:]
                            )
                        # pace the fillers: keep the PE busy while exp runs
                        iters_left = total_iters - it
                        rem = len(s0_q) + len(must_q)
                        need = -(-rem // iters_left)  # ceil
                        n = max(3 if d >= 0 else 2, need)
                        if kb <= 1:
                            # extra cover for the yac-bank handover from the
                            # previous pair's reciprocal readers
                            n = max(n, 4)
                        if it >= 2:
                            drain(n)
                        it += 1
                        if prev is not None:
                            pkb, ppt, poff, pw = prev
                            nc.tensor.matmul(
                                yacA[:, poff:QC], vex[:, pkb, hA, :],
                                ppt[:, 0, :pw],
                                start=(pkb == 0), stop=(pkb == nkb - 1),
                            )
                            nc.tensor.matmul(
                                yacB[:, poff:QC], vex[:, pkb, hB, :],
                                ppt[:, 1, :pw],
                                start=(pkb == 0), stop=(pkb == nkb - 1),
                            )
                        prev = (kb, pt, off, w)
                    drain(3)
                    pkb, ppt, poff, pw = prev
                    nc.tensor.matmul(
                        yacA[:, poff:QC], vex[:, pkb, hA, :], ppt[:, 0, :pw],
                        start=(pkb == 0), stop=(pkb == nkb - 1),
                    )
                    nc.tensor.matmul(
                        yacB[:, poff:QC], vex[:, pkb, hB, :], ppt[:, 1, :pw],
                        start=(pkb == 0), stop=(pkb == nkb - 1),
                    )
                    # stage unnormalized y to SBUF (frees the yac banks) and
                    # kick off the pair's denominator reciprocals
                    yu = yupool.tile([P, QC], BF16, tag="yu", name="yu")
                    nc.vector.tensor_copy(yu[0:HD, :], yacA[:HD, :])
                    nc.vector.tensor_copy(yu[HD:P, :], yacB[:HD, :])
                    # 1/x = exp(-ln(x)) on ScalarE, with the two denominator
                    # rows first gathered into one tile by quick DVE copies
                    # so a SINGLE Ln covers both (ScalarE is the bottleneck
                    # in late chunks). A DVE-reciprocal variant was tried
                    # and REGRESSED ~50us: its 3.35us latency holds the yac
                    # PSUM banks hostage, stalling the next pair's PV
                    # matmuls on the bank handover; the copies here are
                    # ~600ns so yac frees almost immediately.
                    lr, rzb, rg = lrs[g % 2], rzbs[g % 2], rgs[g % 2]
                    nc.vector.tensor_copy(rg[0:1, :], yacA[HD : HD + 1, :])
                    nc.vector.tensor_copy(
                        rg[HD : HD + 1, :], yacB[HD : HD + 1, :]
                    )
                    nc.scalar.activation(
                        lr[:], rg[:], mybir.ActivationFunctionType.Ln,
                    )
                    nc.scalar.activation(
                        rzb[:], lr[:],
                        mybir.ActivationFunctionType.Exp, scale=-1.0,
                    )
                    emit_norm()
                    pending[0] = (qc, ytq, g, yu, rzb)

                # force-drain any stage-1 leftovers before the next chunk's
                # attention needs qt/kt/vex. The last pair's normalization
                # stays pending across the chunk boundary; emit_norm pushes
                # the chunk's proj closures once it fires.
                drain(len(s0_q) + len(must_q))

            emit_norm()
            drain(len(s0_q) + len(must_q) + len(lazy_q))

    return nc


def legalize_waits(nc):
    """This walrus build accepts at most 1 sync wait per instruction (0 for
    self-loading fp32/fp32r Matmult, whose LW slot takes none). Move excess
    waits onto preceding same-engine NoOps; engines execute in order so the
    guarantee is identical."""
    n = 0
    for blk in nc.m.functions[0].blocks:
        new = []
        for inst in blk.instructions:
            si = inst.sync_info
            waits = list(si.on_wait) if si is not None and si.on_wait else []
            lim = 0 if inst.opcode in ("Matmult", "Ldweights") else 1
            if len(waits) > lim:
                keep = waits[len(waits) - lim:] if lim else []
                for w in waits[: len(waits) - lim]:
                    n += 1
                    new.append(mybir.InstNoOp(
                        name=f"I-wfix{n}", engine=inst.engine, ins=[], outs=[],
                        sync_info=mybir.SyncInfo(on_wait=[w], on_update=[]),
                    ))
                inst.sync_info = mybir.SyncInfo(
                    on_wait=keep,
                    on_update=list(si.on_update) if si.on_update else [],
                )
            new.append(inst)
        blk.instructions = new
    return n


def _host_inputs(x, W_attn, b_attn, W_proj):
    """Build the 8 per-core input maps."""
    kl = np.arange(P)[:, None]
    ql = np.arange(P)[None, :]
    mask = (ql >= kl).astype(ml_dtypes.bfloat16)  # [128, 128]

    in_maps = []
    for core in range(8):
        b, g = core // 2, core % 2
        qs = slice(g * NH * HD, (g + 1) * NH * HD)
        ks = slice(C + g * NH * HD, C + (g + 1) * NH * HD)
        vs = slice(2 * C + g * NH * HD, 2 * C + (g + 1) * NH * HD)
        wqk = np.ascontiguousarray(
            np.concatenate([W_attn[:, qs], W_attn[:, ks]], axis=1)
        )
        bqk = (
            np.concatenate([b_attn[qs], b_attn[ks]])
            .reshape(2 * NH * HD // P, P)
            .T.copy()
        )
        in_maps.append(
            {
                "xT": np.ascontiguousarray(x[b].T).astype(ml_dtypes.bfloat16),
                "Wqk": wqk.astype(ml_dtypes.bfloat16),
                "Wv": np.ascontiguousarray(W_attn[:, vs]).astype(
                    ml_dtypes.bfloat16
                ),
                "Wp": np.ascontiguousarray(
                    W_proj[g * NH * HD : (g + 1) * NH * HD]
                ).astype(ml_dtypes.bfloat16),
                "bqk": np.ascontiguousarray(bqk),
                "bv": np.ascontiguousarray(b_attn[vs]),
                "mask": mask,
            }
        )
    return in_maps


def run(x, W_attn, b_attn, W_proj, b_proj, trace=False):
    """Returns (y, BassKernelResults)."""
    x = np.asarray(x, dtype=np.float32)
    W_attn = np.asarray(W_attn, dtype=np.float32)
    b_attn = np.asarray(b_attn, dtype=np.float32)
    W_proj = np.asarray(W_proj, dtype=np.float32)
    b_proj = np.asarray(b_proj, dtype=np.float32)

    nc = build_nc()
    legalize_waits(nc)
    in_maps = _host_inputs(x, W_attn, b_attn, W_proj)
    res = run_bass_kernel_spmd(nc, in_maps, list(range(8)), trace=trace)

    y = np.empty((B, T, C), dtype=np.float32)
    for b in range(B):
        acc = (
            res.results[2 * b]["yT"].astype(np.float32)
            + res.results[2 * b + 1]["yT"].astype(np.float32)
        )
        y[b] = acc.T + b_proj
    return y, res


def kernel(x, W_attn, b_attn, W_proj, b_proj):
    y, _ = run(x, W_attn, b_attn, W_proj, b_proj)
    return y
